# revision 33
# baseline (speedup 1.0000x reference)
"""Distributed QK-norm multi-head attention on 8 Trainium2 NeuronCores.

Sharding: 2-way data parallel on batch x 4-way tensor parallel on heads.
Core c handles batch c//4 and heads 4*(c%4)..4*(c%4)+3. The context
AllGather then runs on 4-core rings with half the bytes per core of a pure
8-way head shard, and only 4 collectives per core, spaced two attention
units apart, so the serialized collective queue never backs up.

All operands are pre-transposed and cast to bf16 on host so every matmul is
in PE-native layout; f32 accumulation; softmax denominators via a
ones-augmented V matmul. The Act engine stays exp-pure (single activation
table): LN rstd is computed on the DVE (Quake rsqrt + Newton), PSUM
evacuations on DVE, LN-apply on GpSimd, softmax normalization via
reciprocal_approx_fast + GpSimd partition broadcast.

kernel(**inputs) takes the full unsharded inputs and returns the full
[2, 2048, 1024] float32 output.
"""

from contextlib import ExitStack

import numpy as np

import concourse.bass as bass
import concourse.bacc as bacc
import concourse.tile as tile
import concourse.mybir as mybir

F32 = mybir.dt.float32
I32 = mybir.dt.int32
BF16 = mybir.dt.bfloat16
AF = mybir.ActivationFunctionType
OP = mybir.AluOpType

N_CORES = 8
B, NSEQ, D = 2, 2048, 1024
H, HD = 16, 64
L = 4                      # tensor-parallel lanes per batch group
HC = H // N_CORES * 2      # heads per core = 4
NHP = HC // 2              # head pairs per core = 2
P = 128
T2 = NSEQ                  # tokens per core (its batch) = 2048
NTB = T2 // P              # 16 token blocks
ND = D // P                # 8 contraction tiles
KB = NSEQ // P             # 16 key blocks
QG = 512                   # q-group (moving free dim)
NQG = NSEQ // QG           # 4 q groups
EPS = 1e-5
WQK = 4 * P                # 512: q+k projection columns
WVM = 2 * P + 2 * HC       # 264: v columns + 8 mean columns
HDP = HD + 1               # V head pitch: 64 dims + ones column


def build(n_cores: int = N_CORES, trivial_gb: bool = True):
    nc = bacc.Bacc("TRN2", target_bir_lowering=False, debug=False,
                   num_devices=n_cores)

    xT = nc.dram_tensor("xT", [D, T2], BF16, kind="ExternalInput")
    wqkT = nc.dram_tensor("wqkT", [D, WQK], BF16, kind="ExternalInput")
    bqk = nc.dram_tensor("bqk", [1, WQK], BF16, kind="ExternalInput")
    wvmT = nc.dram_tensor("wvmT", [D, WVM], BF16, kind="ExternalInput")
    bvm = nc.dram_tensor("bvm", [1, WVM], BF16, kind="ExternalInput")
    wpT = nc.dram_tensor("wpT", [D, 2 * P], BF16, kind="ExternalInput")
    bp = nc.dram_tensor("bp", [2 * P, 1], F32, kind="ExternalInput")
    qg2 = nc.dram_tensor("qg2", [P, 1], F32, kind="ExternalInput")
    qb2 = nc.dram_tensor("qb2", [P, 1], F32, kind="ExternalInput")
    kg2 = nc.dram_tensor("kg2", [P, 1], F32, kind="ExternalInput")
    kb2 = nc.dram_tensor("kb2", [P, 1], F32, kind="ExternalInput")
    ident = nc.dram_tensor("ident", [P, P], BF16, kind="ExternalInput")
    outT = nc.dram_tensor("outT", [2 * P, T2], F32, kind="ExternalOutput")

    groups = [[g * L + i for i in range(L)] for g in range(n_cores // L)]

    with tile.TileContext(nc) as tc, ExitStack() as ctx:
        pools = {}
        for name, bufs, space in [
            ("xt", 1, "SBUF"), ("wq", 1, "SBUF"), ("wp", 1, "SBUF"),
            ("const", 1, "SBUF"), ("qkt", 1, "SBUF"), ("vp", 1, "SBUF"),
            ("raw", 1, "SBUF"), ("stat", 1, "SBUF"), ("sq", 2, "SBUF"),
            ("tok", 4, "SBUF"), ("at", 3, "SBUF"), ("rb", 4, "SBUF"),
            ("cstage", 1, "SBUF"), ("pr", 4, "SBUF"),
            ("osb", 2, "SBUF"), ("dram", 1, "DRAM"),
            ("ps_misc", 2, "PSUM"), ("ps_s", 2, "PSUM"), ("ps_ctx", 2, "PSUM"),
        ]:
            pools[name] = ctx.enter_context(
                tc.tile_pool(name=name, bufs=bufs, space=space))

        # ---- persistent SBUF tensors ----
        xt_all = pools["xt"].tile([P, ND, T2], BF16, name="xt_all")

        def load_xt_chunk(ch, nway=2):
            eng_l = [nc.sync, nc.scalar]
            step = ND // nway
            for i in range(nway):
                src = xT[i * step * P:(i + 1) * step * P,
                         ch * QG:(ch + 1) * QG]
                eng_l[i % 2].dma_start(
                    xt_all[:, i * step:(i + 1) * step,
                           ch * QG:(ch + 1) * QG],
                    src.rearrange("(dt p) q -> p dt q", p=P))

        wqk_all = pools["wq"].tile([P, ND, WQK], BF16, name="wqk_all")
        wvm_all = pools["wq"].tile([P, ND, WVM], BF16, name="wvm_all")

        load_xt_chunk(0, nway=8)
        for dt in range(ND):
            eng = nc.gpsimd if dt % 2 == 0 else nc.scalar
            eng.dma_start(wqk_all[:, dt, :], wqkT[dt * P:(dt + 1) * P, :])
            eng.dma_start(wvm_all[:, dt, :], wvmT[dt * P:(dt + 1) * P, :])
        load_xt_chunk(1)

        wp_all = pools["wp"].tile([P, ND, 2 * P], BF16, name="wp_all")
        nc.gpsimd.dma_start(
            wp_all[:], wpT[:].rearrange("(dt p) q -> p dt q", p=P))

        cp = pools["const"]
        bqk_sb = cp.tile([1, WQK], BF16, name="bqk_sb")
        nc.sync.dma_start(bqk_sb[:], bqk[:])
        bvm_sb = cp.tile([1, WVM], BF16, name="bvm_sb")
        nc.sync.dma_start(bvm_sb[:], bvm[:])
        bp_sb = cp.tile([P, 2], F32, name="bp_sb")
        nc.sync.dma_start(bp_sb[:],
                          bp[:].rearrange("(r p) o -> p (r o)", p=P))
        gb_sb = {}
        for nm, src in (("qg2", qg2), ("qb2", qb2), ("kg2", kg2), ("kb2", kb2)):
            t_ = cp.tile([P, 1], F32, name=f"{nm}_sb")
            nc.sync.dma_start(t_[:], src[:])
            gb_sb[nm] = t_
        ident_sb = cp.tile([P, P], BF16, name="ident_sb")
        nc.sync.dma_start(ident_sb[:], ident[:])
        ones_sb = cp.tile([1, P], BF16, name="ones_sb")
        nc.vector.memset(ones_sb[:], 1.0)
        zero_sb = cp.tile([P, 1], F32, name="zero_sb")
        nc.vector.memset(zero_sb[:], 0.0)

        # per head pair: transposed Q/K [2*64, 2048]
        qt_sb = [pools["qkt"].tile([P, NSEQ], BF16, name=f"qt{hp}")
                 for hp in range(NHP)]
        kt_sb = [pools["qkt"].tile([P, NSEQ], BF16, name=f"kt{hp}")
                 for hp in range(NHP)]
        vp_sb = [pools["vp"].tile([P, KB, 2 * HDP], BF16, name=f"vp{hp}")
                 for hp in range(NHP)]
        for hp in range(NHP):
            for h in range(2):
                nc.vector.memset(
                    vp_sb[hp][:, :, h * HDP + HD:h * HDP + HD + 1], 1.0)
        qkraw = pools["raw"].tile([P, NTB, WQK], BF16, name="qkraw")
        # stats: 8 per token block: [q h0..h3, k h0..h3]
        svar = pools["stat"].tile([P, 8 * NTB], F32, name="svar")
        smu = pools["stat"].tile([P, 8 * NTB], F32, name="smu")
        nmurs = pools["stat"].tile([P, 8 * NTB], F32, name="nmurs")
        rstd_all = pools["stat"].tile([P, 8 * NTB], F32, name="rstd")
        cstage = [pools["cstage"].tile([P, NSEQ], BF16, name=f"cstage{hp}")
                  for hp in range(NHP)]

        warm_in = pools["dram"].tile([P, 4], BF16, name="warm_in")
        warm_out = pools["dram"].tile([P * L, 4], BF16, name="warm_out")
        warm_sb = cp.tile([P, 4], BF16, name="warm_sb")
        nc.vector.memset(warm_sb[:], 0.0)
        nc.sync.dma_start(warm_in[:], warm_sb[:])
        nc.gpsimd.collective_compute(
            "AllGather", OP.bypass, replica_groups=groups,
            ins=[warm_in[:].opt()], outs=[warm_out[:].opt()])

        bounce = [pools["dram"].tile([2 * P, QG], BF16, name=f"bounce{qg}")
                  for qg in range(NQG)]
        gathered = [pools["dram"].tile([2 * P * L, QG], BF16,
                                       name=f"gath{qg}")
                    for qg in range(NQG)]

        QTR = 4                      # token blocks per stats group
        NQTR = NTB // QTR            # 4 quarters

        def b1_quarter(q):
            if q + 2 < NQTR:
                load_xt_chunk(q + 2)
            for tb in range(q * QTR, (q + 1) * QTR):
                psqk = pools["ps_misc"].tile([P, WQK], F32, name="psqk",
                                             tag="misc")
                for dt in range(ND):
                    nc.tensor.matmul(psqk[:],
                                     xt_all[:, dt, tb * P:(tb + 1) * P],
                                     wqk_all[:, dt, :], start=(dt == 0),
                                     stop=False)
                nc.tensor.matmul(psqk[:], ones_sb[:], bqk_sb[:],
                                 start=False, stop=True)
                psvm = pools["ps_misc"].tile([P, WVM], F32, name="psvm",
                                             tag="misc")
                for dt in range(ND):
                    nc.tensor.matmul(psvm[:],
                                     xt_all[:, dt, tb * P:(tb + 1) * P],
                                     wvm_all[:, dt, :], start=(dt == 0),
                                     stop=False)
                nc.tensor.matmul(psvm[:], ones_sb[:], bvm_sb[:],
                                 start=False, stop=True)
                nc.vector.tensor_copy(qkraw[:, tb, :], psqk[:])
                nc.vector.tensor_copy(
                    vp_sb[0][:, tb, :].rearrange("p (h w) -> p h w",
                                                 h=2)[:, :, 0:HD],
                    psvm[:, 0:P].rearrange("p (h w) -> p h w", h=2))
                nc.vector.tensor_copy(
                    vp_sb[1][:, tb, :].rearrange("p (h w) -> p h w",
                                                 h=2)[:, :, 0:HD],
                    psvm[:, P:2 * P].rearrange("p (h w) -> p h w", h=2))
                nc.vector.tensor_copy(smu[:, 8 * tb:8 * tb + 8],
                                      psvm[:, 2 * P:WVM])
                sq = pools["sq"].tile([P, WQK], F32, name="sq", tag="sq")
                nc.vector.tensor_tensor(out=sq[:], in0=qkraw[:, tb, :],
                                        in1=qkraw[:, tb, :], op=OP.mult)
                nc.vector.tensor_reduce(
                    svar[:, 8 * tb:8 * tb + 8],
                    sq[:].rearrange("p (g w) -> p g w", g=8),
                    axis=mybir.AxisListType.X, op=OP.add)
            lo, hi = 8 * q * QTR, 8 * (q + 1) * QTR
            varb = pools["sq"].tile([P, 8 * QTR], F32, name="varb", tag="varb")
            musq = pools["sq"].tile([P, 8 * QTR], F32, name="musq", tag="musq")
            nc.vector.tensor_tensor(out=musq[:], in0=smu[:, lo:hi],
                                    in1=smu[:, lo:hi], op=OP.mult)
            nc.vector.tensor_scalar(varb[:], svar[:, lo:hi], 1.0 / HD, None,
                                    op0=OP.mult)
            nc.vector.tensor_tensor(out=varb[:], in0=varb[:], in1=musq[:],
                                    op=OP.subtract)
            # rstd = rsqrt(var+eps) on DVE (Quake seed + 2 Newton steps)
            nc.vector.tensor_scalar(varb[:], varb[:], EPS, None, op0=OP.add)
            yt = pools["sq"].tile([P, 8 * QTR], F32, name="yt", tag="yt")
            nc.vector.tensor_scalar(
                yt[:].bitcast(I32), varb[:].bitcast(I32), 1, None,
                op0=OP.logical_shift_right)
            nc.vector.tensor_scalar(
                yt[:].bitcast(I32), yt[:].bitcast(I32), -1, 0x5f3759df,
                op0=OP.mult, op1=OP.add)
            for _ in range(2):
                y2 = pools["sq"].tile([P, 8 * QTR], F32, name="y2", tag="y2")
                nc.vector.tensor_tensor(out=y2[:], in0=yt[:], in1=yt[:],
                                        op=OP.mult)
                nc.vector.tensor_tensor(out=y2[:], in0=y2[:], in1=varb[:],
                                        op=OP.mult)
                nc.vector.tensor_scalar(y2[:], y2[:], -0.5, 1.5, op0=OP.mult,
                                        op1=OP.add)
                nc.vector.tensor_tensor(out=yt[:], in0=yt[:], in1=y2[:],
                                        op=OP.mult)
            nc.vector.tensor_copy(rstd_all[:, lo:hi], yt[:])
            nc.vector.tensor_tensor(out=nmurs[:, lo:hi], in0=smu[:, lo:hi],
                                    in1=rstd_all[:, lo:hi], op=OP.mult)
            nc.vector.tensor_scalar(nmurs[:, lo:hi], nmurs[:, lo:hi],
                                    -1.0, None, op0=OP.mult)

        def b2_quarter(hp, q):
            # transpose head pair hp's LN-applied q/k for quarter q
            for tb in range(q * QTR, (q + 1) * QTR):
                for qk, (g2, b2, dst) in enumerate((
                        (gb_sb["qg2"], gb_sb["qb2"], qt_sb[hp]),
                        (gb_sb["kg2"], gb_sb["kb2"], kt_sb[hp]))):
                    tokt = pools["tok"].tile([P, P], BF16, name="tokt",
                                             tag="tok")
                    for hh in range(2):
                        h = 2 * hp + hh
                        i = 8 * tb + 4 * qk + h
                        nc.gpsimd.tensor_scalar(
                            tokt[:, hh * HD:(hh + 1) * HD],
                            qkraw[:, tb,
                                  qk * 2 * P + h * HD:qk * 2 * P + (h + 1) * HD],
                            rstd_all[:, i:i + 1], nmurs[:, i:i + 1],
                            op0=OP.mult, op1=OP.add)
                    pst = pools["ps_misc"].tile([P, P], BF16, name="pst",
                                                tag="misc")
                    nc.tensor.transpose(pst[:], tokt[:], ident_sb[:])
                    if trivial_gb:
                        nc.vector.tensor_copy(
                            dst[:, tb * P:(tb + 1) * P], pst[:])
                    else:
                        nc.vector.tensor_scalar(
                            dst[:, tb * P:(tb + 1) * P], pst[:],
                            g2[:], b2[:], op0=OP.mult, op1=OP.add)

        def scores(hp, qg, kb, pss):
            for h in range(2):
                nc.tensor.matmul(
                    pss[:, h * QG:(h + 1) * QG],
                    kt_sb[hp][h * HD:(h + 1) * HD, kb * P:(kb + 1) * P],
                    qt_sb[hp][h * HD:(h + 1) * HD, qg * QG:(qg + 1) * QG],
                    start=True, stop=True)

        prt_sb = {}

        def issue_prt(qg, nway=1):
            halves = []
            for j in range(2):
                t_ = pools["pr"].tile([P, ND // 2, QG], BF16, name="prt",
                                      tag="pr")
                step = max(ND // 2 // nway, 1)
                for i in range(ND // 2 // step):
                    lo = j * (ND // 2) + i * step
                    src = gathered[qg][lo * P:(lo + step) * P, :]
                    nc.sync.dma_start(
                        t_[:, i * step:(i + 1) * step, :],
                        src.rearrange("(dt p) q -> p dt q", p=P))
                halves.append(t_)
            prt_sb[qg] = halves

        def phase_d(qg):
            halves = prt_sb.pop(qg)
            for r in range(2):
                pso = pools["ps_misc"].tile([P, QG], F32, name="pso",
                                            tag="misc")
                for dt in range(ND):
                    nc.tensor.matmul(
                        pso[:], wp_all[:, dt, r * P:(r + 1) * P],
                        halves[dt // (ND // 2)][:, dt % (ND // 2), :],
                        start=(dt == 0), stop=(dt == ND - 1))
                osb = pools["osb"].tile([P, QG], F32, name="osb", tag="osb")
                nc.vector.tensor_scalar(osb[:], pso[:],
                                        bp_sb[:, r:r + 1], None,
                                        op0=OP.add)
                nc.sync.dma_start(
                    outT[r * P:(r + 1) * P, qg * QG:(qg + 1) * QG], osb[:])

        def phase_c_unit(hp, qg, drains):
            ctx_ps = [pools["ps_ctx"].tile([HD + 1, QG], F32, name="ctx",
                                           tag="ctx") for _ in range(2)]
            pss_tiles = {}
            pss_tiles[0] = pools["ps_s"].tile([P, 2 * QG], F32, name="pss",
                                              tag="pss")
            scores(hp, qg, 0, pss_tiles[0])
            for kb in range(KB):
                if kb + 1 < KB:
                    pss_tiles[kb + 1] = pools["ps_s"].tile(
                        [P, 2 * QG], F32, name="pss", tag="pss")
                    scores(hp, qg, kb + 1, pss_tiles[kb + 1])
                at = pools["at"].tile([P, 2 * QG], BF16, name="at", tag="at")
                nc.scalar.activation(at[:], pss_tiles.pop(kb)[:], AF.Exp,
                                     bias=zero_sb[:], scale=0.125)
                for h in range(2):
                    nc.tensor.matmul(
                        ctx_ps[h][:],
                        vp_sb[hp][:, kb, h * HDP:h * HDP + HD + 1],
                        at[:, h * QG:(h + 1) * QG],
                        start=(kb == 0), stop=(kb == KB - 1))
                if kb == 4 and drains and drains[0] is not None:
                    issue_prt(drains[0][1]) if drains[0][0] == "prt" \
                        else phase_d(drains[0][1])
                if kb == 10 and len(drains) > 1 and drains[1] is not None:
                    issue_prt(drains[1][1]) if drains[1][0] == "prt" \
                        else phase_d(drains[1][1])
            # softmax normalization
            ctxs_l, recs = [], []
            for h in range(2):
                ctxs = pools["rb"].tile([HD + 1, QG], F32, name="ctxs",
                                        tag="ctxs")
                nc.vector.tensor_copy(ctxs[:], ctx_ps[h][0:HD + 1, :])
                ctxs_l.append(ctxs)
                den = pools["rb"].tile([1, QG], F32, name="den", tag="den")
                nc.vector.tensor_copy(den[:], ctxs[HD:HD + 1, :])
                rec = pools["rb"].tile([1, QG], F32, name="rec", tag="rec")
                nc.vector.reciprocal_approx_fast(out=rec[:], in_=den[:])
                recs.append(rec)
            for h in range(2):
                rb = pools["rb"].tile([HD, QG], F32, name="rb", tag="rb")
                nc.gpsimd.partition_broadcast(rb[:], recs[h][:])
                nc.vector.tensor_tensor(
                    out=cstage[hp][h * HD:(h + 1) * HD,
                                   qg * QG:(qg + 1) * QG],
                    in0=ctxs_l[h][0:HD, :], in1=rb[:], op=OP.mult)
            for i in range(2):
                nc.sync.dma_start(
                    bounce[qg][hp * P:(hp + 1) * P,
                               i * (QG // 2):(i + 1) * (QG // 2)],
                    cstage[hp][:, qg * QG + i * (QG // 2):
                               qg * QG + (i + 1) * (QG // 2)])
            if hp == NHP - 1:
                nc.gpsimd.collective_compute(
                    "AllGather", OP.bypass, replica_groups=groups,
                    ins=[bounce[qg][:].opt()],
                    outs=[gathered[qg][:].opt()])

        # head phases: QKV + transposes for both head pairs, lagged one
        # quarter, then the 8 attention units with drains interleaved.
        b1_quarter(0)
        for q in range(1, NQTR):
            b1_quarter(q)
            b2_quarter(0, q - 1)
            b2_quarter(1, q - 1)
        b2_quarter(0, NQTR - 1)
        b2_quarter(1, NQTR - 1)

        sched = {
            (1, 1): [("prt", 0)],
            (0, 2): [("mm", 0)],
            (1, 2): [("prt", 1)],
            (0, 3): [("mm", 1), ("prt", 2)],
            (1, 3): [("mm", 2)],
        }
        for qg in range(NQG):
            phase_c_unit(0, qg, sched.get((0, qg), []))
            phase_c_unit(1, qg, sched.get((1, qg), []))
        issue_prt(3, nway=4)
        phase_d(3)

    nc.compile()
    return nc


def prep_inputs(inputs):
    """Host-side prep: slice/transpose/cast per core. Returns (in_maps, trivial_gb)."""
    import ml_dtypes
    bf16 = ml_dtypes.bfloat16

    q = np.asarray(inputs["query"], np.float32)
    Wq, Wk, Wv, Wp = (np.asarray(inputs[k], np.float32)
                      for k in ("Wq", "Wk", "Wv", "Wp"))
    bq, bk, bv, bpv = (np.asarray(inputs[k], np.float32)
                       for k in ("bq", "bk", "bv", "bp"))
    qg, qb, kg, kb = (np.asarray(inputs[k], np.float32)
                      for k in ("q_gamma", "q_beta", "k_gamma", "k_beta"))

    trivial_gb = bool(
        np.all(qg == 1.0) and np.all(kg == 1.0)
        and np.all(qb == 0.0) and np.all(kb == 0.0))

    identity = np.eye(P, dtype=bf16)
    xTb = [np.ascontiguousarray(q[b].T).astype(bf16) for b in range(B)]
    in_maps = []
    for c in range(N_CORES):
        bt, l = divmod(c, L)
        sl = slice(l * 2 * P, (l + 1) * 2 * P)        # this lane's head dims
        wq_c, wk_c, wv_c = Wq[sl].T, Wk[sl].T, Wv[sl].T   # [1024, 256]
        wqkT = np.concatenate([wq_c, wk_c], axis=1).astype(bf16)
        mean_cols = np.stack(
            [wq_c[:, h * HD:(h + 1) * HD].mean(axis=1) for h in range(HC)]
            + [wk_c[:, h * HD:(h + 1) * HD].mean(axis=1) for h in range(HC)],
            axis=1)                                    # [1024, 8]
        wvmT = np.concatenate([wv_c, mean_cols], axis=1).astype(bf16)
        bq_c, bk_c, bv_c = bq[sl], bk[sl], bv[sl]
        bqk_c = np.concatenate([bq_c, bk_c])[None, :].astype(bf16)
        bias_means = np.array(
            [bq_c[h * HD:(h + 1) * HD].mean() for h in range(HC)]
            + [bk_c[h * HD:(h + 1) * HD].mean() for h in range(HC)],
            np.float32)
        bvm_c = np.concatenate([bv_c, bias_means])[None, :].astype(bf16)
        in_maps.append({
            "xT": xTb[bt],
            "wqkT": np.ascontiguousarray(wqkT),
            "bqk": np.ascontiguousarray(bqk_c),
            "wvmT": np.ascontiguousarray(wvmT),
            "bvm": np.ascontiguousarray(bvm_c),
            "wpT": np.ascontiguousarray(Wp[sl].T).astype(bf16),
            "bp": np.ascontiguousarray(bpv[sl].reshape(2 * P, 1)),
            "qg2": np.tile(qg, 2).reshape(P, 1).astype(np.float32),
            "qb2": np.tile(qb, 2).reshape(P, 1).astype(np.float32),
            "kg2": np.tile(kg, 2).reshape(P, 1).astype(np.float32),
            "kb2": np.tile(kb, 2).reshape(P, 1).astype(np.float32),
            "ident": identity,
        })
    return in_maps, trivial_gb


def assemble_output(results):
    out = np.empty((B, NSEQ, D), np.float32)
    for c in range(N_CORES):
        bt, l = divmod(c, L)
        o = np.asarray(results[c]["outT"], np.float32)   # [256, 2048]
        out[bt, :, l * 2 * P:(l + 1) * 2 * P] = o.T
    return out


_CACHE = {}


def kernel(**inputs):
    from concourse.bass_utils import run_bass_kernel_spmd

    in_maps, trivial = prep_inputs(inputs)
    key = ("nc", trivial)
    if key not in _CACHE:
        _CACHE[key] = build(trivial_gb=trivial)
    nc = _CACHE[key]
    res = run_bass_kernel_spmd(nc, in_maps, core_ids=list(range(N_CORES)))
    return assemble_output(res.results)


# revision 34
# speedup vs baseline: 1.1240x; 1.1240x over previous
"""Distributed QK-norm multi-head attention on 8 Trainium2 NeuronCores.

Sharding: 2-way data parallel on batch x 4-way tensor parallel on heads.
Core c handles batch c//4 and heads 4*(c%4)..4*(c%4)+3. The context
AllGather then runs on 4-core rings with half the bytes per core of a pure
8-way head shard, and only 4 collectives per core, spaced two attention
units apart, so the serialized collective queue never backs up.

All operands are pre-transposed and cast to bf16 on host so every matmul is
in PE-native layout; f32 accumulation; softmax denominators via a
ones-augmented V matmul. The Act engine stays exp-pure (single activation
table): LN rstd is computed on the DVE (Quake rsqrt + Newton), PSUM
evacuations on DVE, LN-apply on GpSimd, softmax normalization via
reciprocal_approx_fast + GpSimd partition broadcast.

kernel(**inputs) takes the full unsharded inputs and returns the full
[2, 2048, 1024] float32 output.
"""

from contextlib import ExitStack

import numpy as np

import concourse.bass as bass
import concourse.bacc as bacc
import concourse.tile as tile
import concourse.mybir as mybir

F32 = mybir.dt.float32
I32 = mybir.dt.int32
BF16 = mybir.dt.bfloat16
AF = mybir.ActivationFunctionType
OP = mybir.AluOpType

N_CORES = 8
B, NSEQ, D = 2, 2048, 1024
H, HD = 16, 64
L = 4                      # tensor-parallel lanes per batch group
HC = H // N_CORES * 2      # heads per core = 4
NHP = HC // 2              # head pairs per core = 2
P = 128
T2 = NSEQ                  # tokens per core (its batch) = 2048
NTB = T2 // P              # 16 token blocks
ND = D // P                # 8 contraction tiles
KB = NSEQ // P             # 16 key blocks
QG = 512                   # q-group (moving free dim)
NQG = NSEQ // QG           # 4 q groups
EPS = 1e-5
WQK = 4 * P                # 512: q+k projection columns
WVM = 2 * P + 2 * HC       # 264: v columns + 8 mean columns
HDP = HD + 1               # V head pitch: 64 dims + ones column


def build(n_cores: int = N_CORES, trivial_gb: bool = True):
    nc = bacc.Bacc("TRN2", target_bir_lowering=False, debug=False,
                   num_devices=n_cores)

    xT = nc.dram_tensor("xT", [D, T2], BF16, kind="ExternalInput")
    wqkT = nc.dram_tensor("wqkT", [D, WQK], BF16, kind="ExternalInput")
    bqk = nc.dram_tensor("bqk", [1, WQK], BF16, kind="ExternalInput")
    wvmT = nc.dram_tensor("wvmT", [D, WVM], BF16, kind="ExternalInput")
    bvm = nc.dram_tensor("bvm", [1, WVM], BF16, kind="ExternalInput")
    wpT = nc.dram_tensor("wpT", [D, 2 * P], BF16, kind="ExternalInput")
    bp = nc.dram_tensor("bp", [2 * P, 1], F32, kind="ExternalInput")
    qg2 = nc.dram_tensor("qg2", [P, 1], F32, kind="ExternalInput")
    qb2 = nc.dram_tensor("qb2", [P, 1], F32, kind="ExternalInput")
    kg2 = nc.dram_tensor("kg2", [P, 1], F32, kind="ExternalInput")
    kb2 = nc.dram_tensor("kb2", [P, 1], F32, kind="ExternalInput")
    ident = nc.dram_tensor("ident", [P, P], BF16, kind="ExternalInput")
    outT = nc.dram_tensor("outT", [2 * P, T2], F32, kind="ExternalOutput")

    groups = [[g * L + i for i in range(L)] for g in range(n_cores // L)]

    with tile.TileContext(nc) as tc, ExitStack() as ctx:
        pools = {}
        for name, bufs, space in [
            ("xt", 1, "SBUF"), ("wq", 1, "SBUF"), ("wp", 1, "SBUF"),
            ("const", 1, "SBUF"), ("qkt", 1, "SBUF"), ("vp", 1, "SBUF"),
            ("raw", 1, "SBUF"), ("stat", 1, "SBUF"), ("sq", 2, "SBUF"),
            ("tok", 4, "SBUF"), ("at", 3, "SBUF"), ("rb", 4, "SBUF"),
            ("cstage", 1, "SBUF"), ("pr", 4, "SBUF"),
            ("osb", 2, "SBUF"), ("dram", 1, "DRAM"),
            ("ps_misc", 2, "PSUM"), ("ps_s", 2, "PSUM"), ("ps_ctx", 2, "PSUM"),
        ]:
            pools[name] = ctx.enter_context(
                tc.tile_pool(name=name, bufs=bufs, space=space))

        # ---- persistent SBUF tensors ----
        xt_all = pools["xt"].tile([P, ND, T2], BF16, name="xt_all")

        def load_xt_chunk(ch, nway=2):
            eng_l = [nc.sync, nc.scalar]
            step = ND // nway
            for i in range(nway):
                src = xT[i * step * P:(i + 1) * step * P,
                         ch * QG:(ch + 1) * QG]
                eng_l[i % 2].dma_start(
                    xt_all[:, i * step:(i + 1) * step,
                           ch * QG:(ch + 1) * QG],
                    src.rearrange("(dt p) q -> p dt q", p=P))

        wqk_all = pools["wq"].tile([P, ND, WQK], BF16, name="wqk_all")
        wvm_all = pools["wq"].tile([P, ND, WVM], BF16, name="wvm_all")

        load_xt_chunk(0, nway=8)
        for dt in range(ND):
            eng = nc.gpsimd if dt % 2 == 0 else nc.scalar
            eng.dma_start(wqk_all[:, dt, :], wqkT[dt * P:(dt + 1) * P, :])
            eng.dma_start(wvm_all[:, dt, :], wvmT[dt * P:(dt + 1) * P, :])
        load_xt_chunk(1)

        wp_all = pools["wp"].tile([P, ND, 2 * P], BF16, name="wp_all")
        nc.gpsimd.dma_start(
            wp_all[:], wpT[:].rearrange("(dt p) q -> p dt q", p=P))

        cp = pools["const"]
        bqk_sb = cp.tile([1, WQK], BF16, name="bqk_sb")
        nc.sync.dma_start(bqk_sb[:], bqk[:])
        bvm_sb = cp.tile([1, WVM], BF16, name="bvm_sb")
        nc.sync.dma_start(bvm_sb[:], bvm[:])
        bp_sb = cp.tile([P, 2], F32, name="bp_sb")
        nc.sync.dma_start(bp_sb[:],
                          bp[:].rearrange("(r p) o -> p (r o)", p=P))
        gb_sb = {}
        for nm, src in (("qg2", qg2), ("qb2", qb2), ("kg2", kg2), ("kb2", kb2)):
            t_ = cp.tile([P, 1], F32, name=f"{nm}_sb")
            nc.sync.dma_start(t_[:], src[:])
            gb_sb[nm] = t_
        ident_sb = cp.tile([P, P], BF16, name="ident_sb")
        nc.sync.dma_start(ident_sb[:], ident[:])
        ones_sb = cp.tile([1, P], BF16, name="ones_sb")
        nc.vector.memset(ones_sb[:], 1.0)
        zero_sb = cp.tile([P, 1], F32, name="zero_sb")
        nc.vector.memset(zero_sb[:], 0.0)

        # per head pair: transposed Q/K [2*64, 2048]
        qt_sb = [pools["qkt"].tile([P, NSEQ], BF16, name=f"qt{hp}")
                 for hp in range(NHP)]
        kt_sb = [pools["qkt"].tile([P, NSEQ], BF16, name=f"kt{hp}")
                 for hp in range(NHP)]
        vp_sb = [pools["vp"].tile([P, KB, 2 * HDP], BF16, name=f"vp{hp}")
                 for hp in range(NHP)]
        for hp in range(NHP):
            for h in range(2):
                nc.vector.memset(
                    vp_sb[hp][:, :, h * HDP + HD:h * HDP + HD + 1], 1.0)
        qkraw = pools["raw"].tile([P, NTB, WQK], BF16, name="qkraw")
        # stats: 8 per token block: [q h0..h3, k h0..h3]
        svar = pools["stat"].tile([P, 8 * NTB], F32, name="svar")
        smu = pools["stat"].tile([P, 8 * NTB], F32, name="smu")
        nmurs = pools["stat"].tile([P, 8 * NTB], F32, name="nmurs")
        rstd_all = pools["stat"].tile([P, 8 * NTB], F32, name="rstd")
        cstage = [pools["cstage"].tile([P, NSEQ], BF16, name=f"cstage{hp}")
                  for hp in range(NHP)]

        warm_in = pools["dram"].tile([P, 4], BF16, name="warm_in")
        warm_out = pools["dram"].tile([P * L, 4], BF16, name="warm_out")
        warm_sb = cp.tile([P, 4], BF16, name="warm_sb")
        nc.vector.memset(warm_sb[:], 0.0)
        nc.sync.dma_start(warm_in[:], warm_sb[:])
        nc.gpsimd.collective_compute(
            "AllGather", OP.bypass, replica_groups=groups,
            ins=[warm_in[:].opt()], outs=[warm_out[:].opt()])

        bounce = [[pools["dram"].tile([P, QG], BF16, name=f"bounce{qg}_{hp}")
                   for hp in range(NHP)] for qg in range(NQG)]
        gathered = [[pools["dram"].tile([P * L, QG], BF16,
                                        name=f"gath{qg}_{hp}")
                     for hp in range(NHP)] for qg in range(NQG)]

        QTR = 4                      # token blocks per stats group
        NQTR = NTB // QTR            # 4 quarters

        def b1_quarter(q):
            if q + 2 < NQTR:
                load_xt_chunk(q + 2)
            for tb in range(q * QTR, (q + 1) * QTR):
                psqk = pools["ps_misc"].tile([P, WQK], F32, name="psqk",
                                             tag="misc")
                for dt in range(ND):
                    nc.tensor.matmul(psqk[:],
                                     xt_all[:, dt, tb * P:(tb + 1) * P],
                                     wqk_all[:, dt, :], start=(dt == 0),
                                     stop=False)
                nc.tensor.matmul(psqk[:], ones_sb[:], bqk_sb[:],
                                 start=False, stop=True)
                nc.vector.tensor_copy(qkraw[:, tb, :], psqk[:])
                sq = pools["sq"].tile([P, WQK], F32, name="sq", tag="sq")
                nc.vector.tensor_tensor(out=sq[:], in0=qkraw[:, tb, :],
                                        in1=qkraw[:, tb, :], op=OP.mult)
                nc.vector.tensor_reduce(
                    svar[:, 8 * tb:8 * tb + 8],
                    sq[:].rearrange("p (g w) -> p g w", g=8),
                    axis=mybir.AxisListType.X, op=OP.add)
            for tb in range(q * QTR, (q + 1) * QTR):
                psvm = pools["ps_misc"].tile([P, WVM], F32, name="psvm",
                                             tag="misc")
                for dt in range(ND):
                    nc.tensor.matmul(psvm[:],
                                     xt_all[:, dt, tb * P:(tb + 1) * P],
                                     wvm_all[:, dt, :], start=(dt == 0),
                                     stop=False)
                nc.tensor.matmul(psvm[:], ones_sb[:], bvm_sb[:],
                                 start=False, stop=True)
                nc.vector.tensor_copy(
                    vp_sb[0][:, tb, :].rearrange("p (h w) -> p h w",
                                                 h=2)[:, :, 0:HD],
                    psvm[:, 0:P].rearrange("p (h w) -> p h w", h=2))
                nc.vector.tensor_copy(
                    vp_sb[1][:, tb, :].rearrange("p (h w) -> p h w",
                                                 h=2)[:, :, 0:HD],
                    psvm[:, P:2 * P].rearrange("p (h w) -> p h w", h=2))
                nc.vector.tensor_copy(smu[:, 8 * tb:8 * tb + 8],
                                      psvm[:, 2 * P:WVM])
            lo, hi = 8 * q * QTR, 8 * (q + 1) * QTR
            varb = pools["sq"].tile([P, 8 * QTR], F32, name="varb", tag="varb")
            musq = pools["sq"].tile([P, 8 * QTR], F32, name="musq", tag="musq")
            nc.vector.tensor_tensor(out=musq[:], in0=smu[:, lo:hi],
                                    in1=smu[:, lo:hi], op=OP.mult)
            nc.vector.tensor_scalar(varb[:], svar[:, lo:hi], 1.0 / HD, None,
                                    op0=OP.mult)
            nc.vector.tensor_tensor(out=varb[:], in0=varb[:], in1=musq[:],
                                    op=OP.subtract)
            # rstd = rsqrt(var+eps) on DVE (Quake seed + 2 Newton steps)
            nc.vector.tensor_scalar(varb[:], varb[:], EPS, None, op0=OP.add)
            yt = pools["sq"].tile([P, 8 * QTR], F32, name="yt", tag="yt")
            nc.vector.tensor_scalar(
                yt[:].bitcast(I32), varb[:].bitcast(I32), 1, None,
                op0=OP.logical_shift_right)
            nc.vector.tensor_scalar(
                yt[:].bitcast(I32), yt[:].bitcast(I32), -1, 0x5f3759df,
                op0=OP.mult, op1=OP.add)
            for _ in range(2):
                y2 = pools["sq"].tile([P, 8 * QTR], F32, name="y2", tag="y2")
                nc.vector.tensor_tensor(out=y2[:], in0=yt[:], in1=yt[:],
                                        op=OP.mult)
                nc.vector.tensor_tensor(out=y2[:], in0=y2[:], in1=varb[:],
                                        op=OP.mult)
                nc.vector.tensor_scalar(y2[:], y2[:], -0.5, 1.5, op0=OP.mult,
                                        op1=OP.add)
                nc.vector.tensor_tensor(out=yt[:], in0=yt[:], in1=y2[:],
                                        op=OP.mult)
            nc.vector.tensor_copy(rstd_all[:, lo:hi], yt[:])
            nc.vector.tensor_tensor(out=nmurs[:, lo:hi], in0=smu[:, lo:hi],
                                    in1=rstd_all[:, lo:hi], op=OP.mult)
            nc.vector.tensor_scalar(nmurs[:, lo:hi], nmurs[:, lo:hi],
                                    -1.0, None, op0=OP.mult)

        def b2_quarter(hp, q):
            # transpose head pair hp's LN-applied q/k for quarter q
            for tb in range(q * QTR, (q + 1) * QTR):
                for qk, (g2, b2, dst) in enumerate((
                        (gb_sb["qg2"], gb_sb["qb2"], qt_sb[hp]),
                        (gb_sb["kg2"], gb_sb["kb2"], kt_sb[hp]))):
                    tokt = pools["tok"].tile([P, P], BF16, name="tokt",
                                             tag="tok")
                    eng = nc.gpsimd if hp == 0 else nc.vector
                    for hh in range(2):
                        h = 2 * hp + hh
                        i = 8 * tb + 4 * qk + h
                        eng.tensor_scalar(
                            tokt[:, hh * HD:(hh + 1) * HD],
                            qkraw[:, tb,
                                  qk * 2 * P + h * HD:qk * 2 * P + (h + 1) * HD],
                            rstd_all[:, i:i + 1], nmurs[:, i:i + 1],
                            op0=OP.mult, op1=OP.add)
                    pst = pools["ps_misc"].tile([P, P], BF16, name="pst",
                                                tag="misc")
                    nc.tensor.transpose(pst[:], tokt[:], ident_sb[:])
                    if trivial_gb:
                        nc.vector.tensor_copy(
                            dst[:, tb * P:(tb + 1) * P], pst[:])
                    else:
                        nc.vector.tensor_scalar(
                            dst[:, tb * P:(tb + 1) * P], pst[:],
                            g2[:], b2[:], op0=OP.mult, op1=OP.add)

        def scores(hp, qg, kb, pss):
            for h in range(2):
                nc.tensor.matmul(
                    pss[:, h * QG:(h + 1) * QG],
                    kt_sb[hp][h * HD:(h + 1) * HD, kb * P:(kb + 1) * P],
                    qt_sb[hp][h * HD:(h + 1) * HD, qg * QG:(qg + 1) * QG],
                    start=True, stop=True)

        prt_sb = {}

        def issue_prt(qg, nway=1):
            # one half-tile per head pair; gathered[qg][hp] rows are lane
            # blocks l*128.., i.e. ctx dim-tiles dt = 2*l + hp
            halves = []
            for hp in range(NHP):
                t_ = pools["pr"].tile([P, L, QG], BF16, name="prt", tag="pr")
                step = max(L // nway, 1)
                for i in range(L // step):
                    src = gathered[qg][hp][i * step * P:(i + 1) * step * P, :]
                    nc.sync.dma_start(
                        t_[:, i * step:(i + 1) * step, :],
                        src.rearrange("(l p) q -> p l q", p=P))
                halves.append(t_)
            prt_sb[qg] = halves

        def phase_d(qg):
            halves = prt_sb.pop(qg)
            for r in range(2):
                pso = pools["ps_misc"].tile([P, QG], F32, name="pso",
                                            tag="misc")
                for dt in range(ND):
                    nc.tensor.matmul(
                        pso[:], wp_all[:, dt, r * P:(r + 1) * P],
                        halves[dt % 2][:, dt // 2, :],
                        start=(dt == 0), stop=(dt == ND - 1))
                osb = pools["osb"].tile([P, QG], F32, name="osb", tag="osb")
                nc.vector.tensor_scalar(osb[:], pso[:],
                                        bp_sb[:, r:r + 1], None,
                                        op0=OP.add)
                for i in range(2):
                    nc.sync.dma_start(
                        outT[r * P:(r + 1) * P,
                             qg * QG + i * (QG // 2):
                             qg * QG + (i + 1) * (QG // 2)],
                        osb[:, i * (QG // 2):(i + 1) * (QG // 2)])

        def phase_c_unit(hp, qg, drains):
            ctx_ps = [pools["ps_ctx"].tile([HD + 1, QG], F32, name="ctx",
                                           tag="ctx") for _ in range(2)]
            pss_tiles = {}
            pss_tiles[0] = pools["ps_s"].tile([P, 2 * QG], F32, name="pss",
                                              tag="pss")
            scores(hp, qg, 0, pss_tiles[0])
            for kb in range(KB):
                if kb + 1 < KB:
                    pss_tiles[kb + 1] = pools["ps_s"].tile(
                        [P, 2 * QG], F32, name="pss", tag="pss")
                    scores(hp, qg, kb + 1, pss_tiles[kb + 1])
                at = pools["at"].tile([P, 2 * QG], BF16, name="at", tag="at")
                nc.scalar.activation(at[:], pss_tiles.pop(kb)[:], AF.Exp,
                                     bias=zero_sb[:], scale=0.125)
                for h in range(2):
                    nc.tensor.matmul(
                        ctx_ps[h][:],
                        vp_sb[hp][:, kb, h * HDP:h * HDP + HD + 1],
                        at[:, h * QG:(h + 1) * QG],
                        start=(kb == 0), stop=(kb == KB - 1))
                if kb == 4 and drains and drains[0] is not None:
                    issue_prt(drains[0][1]) if drains[0][0] == "prt" \
                        else phase_d(drains[0][1])
                if kb == 10 and len(drains) > 1 and drains[1] is not None:
                    issue_prt(drains[1][1]) if drains[1][0] == "prt" \
                        else phase_d(drains[1][1])
            # softmax normalization
            ctxs_l, recs = [], []
            for h in range(2):
                ctxs = pools["rb"].tile([HD + 1, QG], F32, name="ctxs",
                                        tag="ctxs")
                nc.vector.tensor_copy(ctxs[:], ctx_ps[h][0:HD + 1, :])
                ctxs_l.append(ctxs)
                den = pools["rb"].tile([1, QG], F32, name="den", tag="den")
                nc.vector.tensor_copy(den[:], ctxs[HD:HD + 1, :])
                rec = pools["rb"].tile([1, QG], F32, name="rec", tag="rec")
                nc.vector.reciprocal_approx_fast(out=rec[:], in_=den[:])
                recs.append(rec)
            for h in range(2):
                rb = pools["rb"].tile([HD, QG], F32, name="rb", tag="rb")
                nc.gpsimd.partition_broadcast(rb[:], recs[h][:])
                nc.vector.tensor_tensor(
                    out=cstage[hp][h * HD:(h + 1) * HD,
                                   qg * QG:(qg + 1) * QG],
                    in0=ctxs_l[h][0:HD, :], in1=rb[:], op=OP.mult)
            for i in range(2):
                nc.sync.dma_start(
                    bounce[qg][hp][:, i * (QG // 2):(i + 1) * (QG // 2)],
                    cstage[hp][:, qg * QG + i * (QG // 2):
                               qg * QG + (i + 1) * (QG // 2)])
            nc.gpsimd.collective_compute(
                "AllGather", OP.bypass, replica_groups=groups,
                ins=[bounce[qg][hp][:].opt()],
                outs=[gathered[qg][hp][:].opt()])

        # head phases: QKV + transposes for both head pairs, lagged one
        # quarter, then the 8 attention units with drains interleaved.
        b1_quarter(0)
        for q in range(1, NQTR):
            b1_quarter(q)
            b2_quarter(0, q - 1)
            b2_quarter(1, q - 1)
        b2_quarter(0, NQTR - 1)
        b2_quarter(1, NQTR - 1)

        sched = {
            (1, 1): [("prt", 0)],
            (0, 2): [("mm", 0)],
            (1, 2): [("prt", 1)],
            (0, 3): [("mm", 1), ("prt", 2)],
            (1, 3): [("mm", 2)],
        }
        for qg in range(NQG):
            phase_c_unit(0, qg, sched.get((0, qg), []))
            phase_c_unit(1, qg, sched.get((1, qg), []))
        issue_prt(3, nway=4)
        phase_d(3)

    nc.compile()
    return nc


def prep_inputs(inputs):
    """Host-side prep: slice/transpose/cast per core. Returns (in_maps, trivial_gb)."""
    import ml_dtypes
    bf16 = ml_dtypes.bfloat16

    q = np.asarray(inputs["query"], np.float32)
    Wq, Wk, Wv, Wp = (np.asarray(inputs[k], np.float32)
                      for k in ("Wq", "Wk", "Wv", "Wp"))
    bq, bk, bv, bpv = (np.asarray(inputs[k], np.float32)
                       for k in ("bq", "bk", "bv", "bp"))
    qg, qb, kg, kb = (np.asarray(inputs[k], np.float32)
                      for k in ("q_gamma", "q_beta", "k_gamma", "k_beta"))

    trivial_gb = bool(
        np.all(qg == 1.0) and np.all(kg == 1.0)
        and np.all(qb == 0.0) and np.all(kb == 0.0))

    identity = np.eye(P, dtype=bf16)
    xTb = [np.ascontiguousarray(q[b].T).astype(bf16) for b in range(B)]
    in_maps = []
    for c in range(N_CORES):
        bt, l = divmod(c, L)
        sl = slice(l * 2 * P, (l + 1) * 2 * P)        # this lane's head dims
        wq_c, wk_c, wv_c = Wq[sl].T, Wk[sl].T, Wv[sl].T   # [1024, 256]
        wqkT = np.concatenate([wq_c, wk_c], axis=1).astype(bf16)
        mean_cols = np.stack(
            [wq_c[:, h * HD:(h + 1) * HD].mean(axis=1) for h in range(HC)]
            + [wk_c[:, h * HD:(h + 1) * HD].mean(axis=1) for h in range(HC)],
            axis=1)                                    # [1024, 8]
        wvmT = np.concatenate([wv_c, mean_cols], axis=1).astype(bf16)
        bq_c, bk_c, bv_c = bq[sl], bk[sl], bv[sl]
        bqk_c = np.concatenate([bq_c, bk_c])[None, :].astype(bf16)
        bias_means = np.array(
            [bq_c[h * HD:(h + 1) * HD].mean() for h in range(HC)]
            + [bk_c[h * HD:(h + 1) * HD].mean() for h in range(HC)],
            np.float32)
        bvm_c = np.concatenate([bv_c, bias_means])[None, :].astype(bf16)
        in_maps.append({
            "xT": xTb[bt],
            "wqkT": np.ascontiguousarray(wqkT),
            "bqk": np.ascontiguousarray(bqk_c),
            "wvmT": np.ascontiguousarray(wvmT),
            "bvm": np.ascontiguousarray(bvm_c),
            "wpT": np.ascontiguousarray(Wp[sl].T).astype(bf16),
            "bp": np.ascontiguousarray(bpv[sl].reshape(2 * P, 1)),
            "qg2": np.tile(qg, 2).reshape(P, 1).astype(np.float32),
            "qb2": np.tile(qb, 2).reshape(P, 1).astype(np.float32),
            "kg2": np.tile(kg, 2).reshape(P, 1).astype(np.float32),
            "kb2": np.tile(kb, 2).reshape(P, 1).astype(np.float32),
            "ident": identity,
        })
    return in_maps, trivial_gb


def assemble_output(results):
    out = np.empty((B, NSEQ, D), np.float32)
    for c in range(N_CORES):
        bt, l = divmod(c, L)
        o = np.asarray(results[c]["outT"], np.float32)   # [256, 2048]
        out[bt, :, l * 2 * P:(l + 1) * 2 * P] = o.T
    return out


_CACHE = {}


def kernel(**inputs):
    from concourse.bass_utils import run_bass_kernel_spmd

    in_maps, trivial = prep_inputs(inputs)
    key = ("nc", trivial)
    if key not in _CACHE:
        _CACHE[key] = build(trivial_gb=trivial)
    nc = _CACHE[key]
    res = run_bass_kernel_spmd(nc, in_maps, core_ids=list(range(N_CORES)))
    return assemble_output(res.results)


# revision 35
# speedup vs baseline: 1.1271x; 1.0027x over previous
"""Distributed QK-norm multi-head attention on 8 Trainium2 NeuronCores.

Sharding: 2-way data parallel on batch x 4-way tensor parallel on heads.
Core c handles batch c//4 and heads 4*(c%4)..4*(c%4)+3. The context
AllGather then runs on 4-core rings with half the bytes per core of a pure
8-way head shard, and only 4 collectives per core, spaced two attention
units apart, so the serialized collective queue never backs up.

All operands are pre-transposed and cast to bf16 on host so every matmul is
in PE-native layout; f32 accumulation; softmax denominators via a
ones-augmented V matmul. The Act engine stays exp-pure (single activation
table): LN rstd is computed on the DVE (Quake rsqrt + Newton), PSUM
evacuations on DVE, LN-apply on GpSimd, softmax normalization via
reciprocal_approx_fast + GpSimd partition broadcast.

kernel(**inputs) takes the full unsharded inputs and returns the full
[2, 2048, 1024] float32 output.
"""

from contextlib import ExitStack

import numpy as np

import concourse.bass as bass
import concourse.bacc as bacc
import concourse.tile as tile
import concourse.mybir as mybir

F32 = mybir.dt.float32
I32 = mybir.dt.int32
BF16 = mybir.dt.bfloat16
AF = mybir.ActivationFunctionType
OP = mybir.AluOpType

N_CORES = 8
B, NSEQ, D = 2, 2048, 1024
H, HD = 16, 64
L = 4                      # tensor-parallel lanes per batch group
HC = H // N_CORES * 2      # heads per core = 4
NHP = HC // 2              # head pairs per core = 2
P = 128
T2 = NSEQ                  # tokens per core (its batch) = 2048
NTB = T2 // P              # 16 token blocks
ND = D // P                # 8 contraction tiles
KB = NSEQ // P             # 16 key blocks
QG = 512                   # q-group (moving free dim)
NQG = NSEQ // QG           # 4 q groups
EPS = 1e-5
WQK = 4 * P                # 512: q+k projection columns
WVM = 2 * P + 2 * HC       # 264: v columns + 8 mean columns
HDP = HD + 1               # V head pitch: 64 dims + ones column


def build(n_cores: int = N_CORES, trivial_gb: bool = True):
    nc = bacc.Bacc("TRN2", target_bir_lowering=False, debug=False,
                   num_devices=n_cores)

    xT = nc.dram_tensor("xT", [D, T2], BF16, kind="ExternalInput")
    wqkT = nc.dram_tensor("wqkT", [D, WQK], BF16, kind="ExternalInput")
    bqk = nc.dram_tensor("bqk", [1, WQK], BF16, kind="ExternalInput")
    wvmT = nc.dram_tensor("wvmT", [D, WVM], BF16, kind="ExternalInput")
    bvm = nc.dram_tensor("bvm", [1, WVM], BF16, kind="ExternalInput")
    wpT = nc.dram_tensor("wpT", [D, 2 * P], BF16, kind="ExternalInput")
    bp = nc.dram_tensor("bp", [2 * P, 1], F32, kind="ExternalInput")
    qg2 = nc.dram_tensor("qg2", [P, 1], F32, kind="ExternalInput")
    qb2 = nc.dram_tensor("qb2", [P, 1], F32, kind="ExternalInput")
    kg2 = nc.dram_tensor("kg2", [P, 1], F32, kind="ExternalInput")
    kb2 = nc.dram_tensor("kb2", [P, 1], F32, kind="ExternalInput")
    ident = nc.dram_tensor("ident", [P, P], BF16, kind="ExternalInput")
    outT = nc.dram_tensor("outT", [2 * P, T2], F32, kind="ExternalOutput")

    groups = [[g * L + i for i in range(L)] for g in range(n_cores // L)]

    with tile.TileContext(nc) as tc, ExitStack() as ctx:
        pools = {}
        for name, bufs, space in [
            ("xt", 1, "SBUF"), ("wq", 1, "SBUF"), ("wp", 1, "SBUF"),
            ("const", 1, "SBUF"), ("qkt", 1, "SBUF"), ("vp", 1, "SBUF"),
            ("raw", 1, "SBUF"), ("stat", 1, "SBUF"), ("sq", 2, "SBUF"),
            ("tok", 4, "SBUF"), ("at", 3, "SBUF"), ("rb", 4, "SBUF"),
            ("cstage", 1, "SBUF"), ("pr", 4, "SBUF"),
            ("osb", 2, "SBUF"), ("dram", 1, "DRAM"),
            ("ps_misc", 2, "PSUM"), ("ps_s", 2, "PSUM"), ("ps_ctx", 2, "PSUM"),
        ]:
            pools[name] = ctx.enter_context(
                tc.tile_pool(name=name, bufs=bufs, space=space))

        # ---- persistent SBUF tensors ----
        xt_all = pools["xt"].tile([P, ND, T2], BF16, name="xt_all")

        def load_xt_chunk(ch, nway=2):
            eng_l = [nc.sync, nc.scalar]
            step = ND // nway
            for i in range(nway):
                src = xT[i * step * P:(i + 1) * step * P,
                         ch * QG:(ch + 1) * QG]
                eng_l[i % 2].dma_start(
                    xt_all[:, i * step:(i + 1) * step,
                           ch * QG:(ch + 1) * QG],
                    src.rearrange("(dt p) q -> p dt q", p=P))

        wqk_all = pools["wq"].tile([P, ND, WQK], BF16, name="wqk_all")
        wvm_all = pools["wq"].tile([P, ND, WVM], BF16, name="wvm_all")

        load_xt_chunk(0, nway=8)
        for dt in range(ND):
            eng = nc.gpsimd if dt % 2 == 0 else nc.scalar
            eng.dma_start(wqk_all[:, dt, :], wqkT[dt * P:(dt + 1) * P, :])
            eng.dma_start(wvm_all[:, dt, :], wvmT[dt * P:(dt + 1) * P, :])
        load_xt_chunk(1)

        wp_all = pools["wp"].tile([P, ND, 2 * P], BF16, name="wp_all")
        nc.gpsimd.dma_start(
            wp_all[:], wpT[:].rearrange("(dt p) q -> p dt q", p=P))

        cp = pools["const"]
        bqk_sb = cp.tile([1, WQK], BF16, name="bqk_sb")
        nc.sync.dma_start(bqk_sb[:], bqk[:])
        bvm_sb = cp.tile([1, WVM], BF16, name="bvm_sb")
        nc.sync.dma_start(bvm_sb[:], bvm[:])
        bp_sb = cp.tile([P, 2], F32, name="bp_sb")
        nc.sync.dma_start(bp_sb[:],
                          bp[:].rearrange("(r p) o -> p (r o)", p=P))
        gb_sb = {}
        for nm, src in (("qg2", qg2), ("qb2", qb2), ("kg2", kg2), ("kb2", kb2)):
            t_ = cp.tile([P, 1], F32, name=f"{nm}_sb")
            nc.sync.dma_start(t_[:], src[:])
            gb_sb[nm] = t_
        ident_sb = cp.tile([P, P], BF16, name="ident_sb")
        nc.sync.dma_start(ident_sb[:], ident[:])
        ones_sb = cp.tile([1, P], BF16, name="ones_sb")
        nc.vector.memset(ones_sb[:], 1.0)
        zero_sb = cp.tile([P, 1], F32, name="zero_sb")
        nc.vector.memset(zero_sb[:], 0.0)

        # per head pair: transposed Q/K [2*64, 2048]
        qt_sb = [pools["qkt"].tile([P, NSEQ], BF16, name=f"qt{hp}")
                 for hp in range(NHP)]
        kt_sb = [pools["qkt"].tile([P, NSEQ], BF16, name=f"kt{hp}")
                 for hp in range(NHP)]
        vp_sb = [pools["vp"].tile([P, KB, 2 * HDP], BF16, name=f"vp{hp}")
                 for hp in range(NHP)]
        for hp in range(NHP):
            for h in range(2):
                nc.vector.memset(
                    vp_sb[hp][:, :, h * HDP + HD:h * HDP + HD + 1], 1.0)
        qkraw = pools["raw"].tile([P, NTB, WQK], BF16, name="qkraw")
        # stats: 8 per token block: [q h0..h3, k h0..h3]
        svar = pools["stat"].tile([P, 8 * NTB], F32, name="svar")
        smu = pools["stat"].tile([P, 8 * NTB], F32, name="smu")
        nmurs = pools["stat"].tile([P, 8 * NTB], F32, name="nmurs")
        rstd_all = pools["stat"].tile([P, 8 * NTB], F32, name="rstd")
        cstage = [pools["cstage"].tile([P, NSEQ], BF16, name=f"cstage{hp}")
                  for hp in range(NHP)]

        warm_in = pools["dram"].tile([P, 4], BF16, name="warm_in")
        warm_out = pools["dram"].tile([P * L, 4], BF16, name="warm_out")
        warm_sb = cp.tile([P, 4], BF16, name="warm_sb")
        nc.vector.memset(warm_sb[:], 0.0)
        nc.sync.dma_start(warm_in[:], warm_sb[:])
        nc.gpsimd.collective_compute(
            "AllGather", OP.bypass, replica_groups=groups,
            ins=[warm_in[:].opt()], outs=[warm_out[:].opt()])

        bounce = [[pools["dram"].tile([P, QG], BF16, name=f"bounce{qg}_{hp}")
                   for hp in range(NHP)] for qg in range(NQG)]
        gathered = [[pools["dram"].tile([P * L, QG], BF16,
                                        name=f"gath{qg}_{hp}")
                     for hp in range(NHP)] for qg in range(NQG)]

        QTR = 4                      # token blocks per stats group
        NQTR = NTB // QTR            # 4 quarters

        def b1_quarter(q):
            if q + 2 < NQTR:
                load_xt_chunk(q + 2)
            for tb in range(q * QTR, (q + 1) * QTR):
                psqk = pools["ps_misc"].tile([P, WQK], F32, name="psqk",
                                             tag="misc")
                for dt in range(ND):
                    nc.tensor.matmul(psqk[:],
                                     xt_all[:, dt, tb * P:(tb + 1) * P],
                                     wqk_all[:, dt, :], start=(dt == 0),
                                     stop=False)
                nc.tensor.matmul(psqk[:], ones_sb[:], bqk_sb[:],
                                 start=False, stop=True)
                nc.vector.tensor_copy(qkraw[:, tb, :], psqk[:])
                sq = pools["sq"].tile([P, WQK], F32, name="sq", tag="sq")
                nc.vector.tensor_tensor(out=sq[:], in0=qkraw[:, tb, :],
                                        in1=qkraw[:, tb, :], op=OP.mult)
                nc.vector.tensor_reduce(
                    svar[:, 8 * tb:8 * tb + 8],
                    sq[:].rearrange("p (g w) -> p g w", g=8),
                    axis=mybir.AxisListType.X, op=OP.add)
            for tb in range(q * QTR, (q + 1) * QTR):
                psvm = pools["ps_misc"].tile([P, WVM], F32, name="psvm",
                                             tag="misc")
                for dt in range(ND):
                    nc.tensor.matmul(psvm[:],
                                     xt_all[:, dt, tb * P:(tb + 1) * P],
                                     wvm_all[:, dt, :], start=(dt == 0),
                                     stop=False)
                nc.tensor.matmul(psvm[:], ones_sb[:], bvm_sb[:],
                                 start=False, stop=True)
                nc.vector.tensor_copy(
                    vp_sb[0][:, tb, :].rearrange("p (h w) -> p h w",
                                                 h=2)[:, :, 0:HD],
                    psvm[:, 0:P].rearrange("p (h w) -> p h w", h=2))
                nc.vector.tensor_copy(
                    vp_sb[1][:, tb, :].rearrange("p (h w) -> p h w",
                                                 h=2)[:, :, 0:HD],
                    psvm[:, P:2 * P].rearrange("p (h w) -> p h w", h=2))
                nc.vector.tensor_copy(smu[:, 8 * tb:8 * tb + 8],
                                      psvm[:, 2 * P:WVM])
            lo, hi = 8 * q * QTR, 8 * (q + 1) * QTR
            varb = pools["sq"].tile([P, 8 * QTR], F32, name="varb", tag="varb")
            musq = pools["sq"].tile([P, 8 * QTR], F32, name="musq", tag="musq")
            nc.vector.tensor_tensor(out=musq[:], in0=smu[:, lo:hi],
                                    in1=smu[:, lo:hi], op=OP.mult)
            nc.vector.tensor_scalar(varb[:], svar[:, lo:hi], 1.0 / HD, None,
                                    op0=OP.mult)
            nc.vector.tensor_tensor(out=varb[:], in0=varb[:], in1=musq[:],
                                    op=OP.subtract)
            # rstd = rsqrt(var+eps) on DVE (Quake seed + 2 Newton steps)
            nc.vector.tensor_scalar(varb[:], varb[:], EPS, None, op0=OP.add)
            yt = pools["sq"].tile([P, 8 * QTR], F32, name="yt", tag="yt")
            nc.vector.tensor_scalar(
                yt[:].bitcast(I32), varb[:].bitcast(I32), 1, None,
                op0=OP.logical_shift_right)
            nc.vector.tensor_scalar(
                yt[:].bitcast(I32), yt[:].bitcast(I32), -1, 0x5f3759df,
                op0=OP.mult, op1=OP.add)
            for _ in range(2):
                y2 = pools["sq"].tile([P, 8 * QTR], F32, name="y2", tag="y2")
                nc.vector.tensor_tensor(out=y2[:], in0=yt[:], in1=yt[:],
                                        op=OP.mult)
                nc.vector.tensor_tensor(out=y2[:], in0=y2[:], in1=varb[:],
                                        op=OP.mult)
                nc.vector.tensor_scalar(y2[:], y2[:], -0.5, 1.5, op0=OP.mult,
                                        op1=OP.add)
                nc.vector.tensor_tensor(out=yt[:], in0=yt[:], in1=y2[:],
                                        op=OP.mult)
            nc.vector.tensor_copy(rstd_all[:, lo:hi], yt[:])
            nc.vector.tensor_tensor(out=nmurs[:, lo:hi], in0=smu[:, lo:hi],
                                    in1=rstd_all[:, lo:hi], op=OP.mult)
            nc.vector.tensor_scalar(nmurs[:, lo:hi], nmurs[:, lo:hi],
                                    -1.0, None, op0=OP.mult)

        def b2_quarter(hp, q):
            # transpose head pair hp's LN-applied q/k for quarter q
            for tb in range(q * QTR, (q + 1) * QTR):
                for qk, (g2, b2, dst) in enumerate((
                        (gb_sb["qg2"], gb_sb["qb2"], qt_sb[hp]),
                        (gb_sb["kg2"], gb_sb["kb2"], kt_sb[hp]))):
                    tokt = pools["tok"].tile([P, P], BF16, name="tokt",
                                             tag="tok")
                    eng = nc.gpsimd if hp == 0 else nc.vector
                    for hh in range(2):
                        h = 2 * hp + hh
                        i = 8 * tb + 4 * qk + h
                        eng.tensor_scalar(
                            tokt[:, hh * HD:(hh + 1) * HD],
                            qkraw[:, tb,
                                  qk * 2 * P + h * HD:qk * 2 * P + (h + 1) * HD],
                            rstd_all[:, i:i + 1], nmurs[:, i:i + 1],
                            op0=OP.mult, op1=OP.add)
                    pst = pools["ps_misc"].tile([P, P], BF16, name="pst",
                                                tag="misc")
                    nc.tensor.transpose(pst[:], tokt[:], ident_sb[:])
                    if trivial_gb:
                        nc.vector.tensor_copy(
                            dst[:, tb * P:(tb + 1) * P], pst[:])
                    else:
                        nc.vector.tensor_scalar(
                            dst[:, tb * P:(tb + 1) * P], pst[:],
                            g2[:], b2[:], op0=OP.mult, op1=OP.add)

        def scores(hp, qg, kb, pss):
            for h in range(2):
                nc.tensor.matmul(
                    pss[:, h * QG:(h + 1) * QG],
                    kt_sb[hp][h * HD:(h + 1) * HD, kb * P:(kb + 1) * P],
                    qt_sb[hp][h * HD:(h + 1) * HD, qg * QG:(qg + 1) * QG],
                    start=True, stop=True)

        prt_sb = {}

        def issue_prt(qg, nway=1, hps=(0, 1)):
            # one half-tile per head pair; gathered[qg][hp] rows are lane
            # blocks l*128.., i.e. ctx dim-tiles dt = 2*l + hp
            halves = prt_sb.setdefault(qg, {})
            for hp in hps:
                t_ = pools["pr"].tile([P, L, QG], BF16, name="prt", tag="pr")
                step = max(L // nway, 1)
                for i in range(L // step):
                    src = gathered[qg][hp][i * step * P:(i + 1) * step * P, :]
                    nc.sync.dma_start(
                        t_[:, i * step:(i + 1) * step, :],
                        src.rearrange("(l p) q -> p l q", p=P))
                halves[hp] = t_

        def phase_d(qg):
            halves = prt_sb.pop(qg)
            for r in range(2):
                pso = pools["ps_misc"].tile([P, QG], F32, name="pso",
                                            tag="misc")
                dts = [0, 2, 4, 6, 1, 3, 5, 7]
                for k, dt in enumerate(dts):
                    nc.tensor.matmul(
                        pso[:], wp_all[:, dt, r * P:(r + 1) * P],
                        halves[dt % 2][:, dt // 2, :],
                        start=(k == 0), stop=(k == ND - 1))
                osb = pools["osb"].tile([P, QG], F32, name="osb", tag="osb")
                nc.vector.tensor_scalar(osb[:], pso[:],
                                        bp_sb[:, r:r + 1], None,
                                        op0=OP.add)
                for i in range(2):
                    nc.sync.dma_start(
                        outT[r * P:(r + 1) * P,
                             qg * QG + i * (QG // 2):
                             qg * QG + (i + 1) * (QG // 2)],
                        osb[:, i * (QG // 2):(i + 1) * (QG // 2)])

        def phase_c_unit(hp, qg, drains):
            ctx_ps = [pools["ps_ctx"].tile([HD + 1, QG], F32, name="ctx",
                                           tag="ctx") for _ in range(2)]
            pss_tiles = {}
            pss_tiles[0] = pools["ps_s"].tile([P, 2 * QG], F32, name="pss",
                                              tag="pss")
            scores(hp, qg, 0, pss_tiles[0])
            for kb in range(KB):
                if kb + 1 < KB:
                    pss_tiles[kb + 1] = pools["ps_s"].tile(
                        [P, 2 * QG], F32, name="pss", tag="pss")
                    scores(hp, qg, kb + 1, pss_tiles[kb + 1])
                at = pools["at"].tile([P, 2 * QG], BF16, name="at", tag="at")
                nc.scalar.activation(at[:], pss_tiles.pop(kb)[:], AF.Exp,
                                     bias=zero_sb[:], scale=0.125)
                for h in range(2):
                    nc.tensor.matmul(
                        ctx_ps[h][:],
                        vp_sb[hp][:, kb, h * HDP:h * HDP + HD + 1],
                        at[:, h * QG:(h + 1) * QG],
                        start=(kb == 0), stop=(kb == KB - 1))
                for slot, kbpos in ((0, 4), (1, 10)):
                    if kb == kbpos and len(drains) > slot:
                        kind, arg = drains[slot]
                        if kind == "prt":
                            issue_prt(arg)
                        elif kind == "prt3a":
                            issue_prt(3, nway=4, hps=(0,))
                        else:
                            phase_d(arg)
            # softmax normalization
            ctxs_l, recs = [], []
            for h in range(2):
                ctxs = pools["rb"].tile([HD + 1, QG], F32, name="ctxs",
                                        tag="ctxs")
                nc.vector.tensor_copy(ctxs[:], ctx_ps[h][0:HD + 1, :])
                ctxs_l.append(ctxs)
                den = pools["rb"].tile([1, QG], F32, name="den", tag="den")
                nc.vector.tensor_copy(den[:], ctxs[HD:HD + 1, :])
                rec = pools["rb"].tile([1, QG], F32, name="rec", tag="rec")
                nc.vector.reciprocal_approx_fast(out=rec[:], in_=den[:])
                recs.append(rec)
            for h in range(2):
                rb = pools["rb"].tile([HD, QG], F32, name="rb", tag="rb")
                nc.gpsimd.partition_broadcast(rb[:], recs[h][:])
                nc.vector.tensor_tensor(
                    out=cstage[hp][h * HD:(h + 1) * HD,
                                   qg * QG:(qg + 1) * QG],
                    in0=ctxs_l[h][0:HD, :], in1=rb[:], op=OP.mult)
            for i in range(2):
                nc.sync.dma_start(
                    bounce[qg][hp][:, i * (QG // 2):(i + 1) * (QG // 2)],
                    cstage[hp][:, qg * QG + i * (QG // 2):
                               qg * QG + (i + 1) * (QG // 2)])
            nc.gpsimd.collective_compute(
                "AllGather", OP.bypass, replica_groups=groups,
                ins=[bounce[qg][hp][:].opt()],
                outs=[gathered[qg][hp][:].opt()])

        # head phases: QKV + transposes for both head pairs, lagged one
        # quarter, then the 8 attention units with drains interleaved.
        b1_quarter(0)
        for q in range(1, NQTR):
            b1_quarter(q)
            b2_quarter(0, q - 1)
            b2_quarter(1, q - 1)
        b2_quarter(0, NQTR - 1)
        b2_quarter(1, NQTR - 1)

        sched = {
            (1, 1): [("prt", 0)],
            (0, 2): [("mm", 0)],
            (1, 2): [("prt", 1)],
            (0, 3): [("mm", 1), ("prt", 2)],
            (1, 3): [("mm", 2)],
        }
        sched[(1, 3)] = sched[(1, 3)] + [("prt3a", None)]
        for qg in range(NQG):
            phase_c_unit(0, qg, sched.get((0, qg), []))
            phase_c_unit(1, qg, sched.get((1, qg), []))
        issue_prt(3, nway=4, hps=(1,))
        phase_d(3)

    nc.compile()
    return nc


def prep_inputs(inputs):
    """Host-side prep: slice/transpose/cast per core. Returns (in_maps, trivial_gb)."""
    import ml_dtypes
    bf16 = ml_dtypes.bfloat16

    q = np.asarray(inputs["query"], np.float32)
    Wq, Wk, Wv, Wp = (np.asarray(inputs[k], np.float32)
                      for k in ("Wq", "Wk", "Wv", "Wp"))
    bq, bk, bv, bpv = (np.asarray(inputs[k], np.float32)
                       for k in ("bq", "bk", "bv", "bp"))
    qg, qb, kg, kb = (np.asarray(inputs[k], np.float32)
                      for k in ("q_gamma", "q_beta", "k_gamma", "k_beta"))

    trivial_gb = bool(
        np.all(qg == 1.0) and np.all(kg == 1.0)
        and np.all(qb == 0.0) and np.all(kb == 0.0))

    identity = np.eye(P, dtype=bf16)
    xTb = [np.ascontiguousarray(q[b].T).astype(bf16) for b in range(B)]
    in_maps = []
    for c in range(N_CORES):
        bt, l = divmod(c, L)
        sl = slice(l * 2 * P, (l + 1) * 2 * P)        # this lane's head dims
        wq_c, wk_c, wv_c = Wq[sl].T, Wk[sl].T, Wv[sl].T   # [1024, 256]
        wqkT = np.concatenate([wq_c, wk_c], axis=1).astype(bf16)
        mean_cols = np.stack(
            [wq_c[:, h * HD:(h + 1) * HD].mean(axis=1) for h in range(HC)]
            + [wk_c[:, h * HD:(h + 1) * HD].mean(axis=1) for h in range(HC)],
            axis=1)                                    # [1024, 8]
        wvmT = np.concatenate([wv_c, mean_cols], axis=1).astype(bf16)
        bq_c, bk_c, bv_c = bq[sl], bk[sl], bv[sl]
        bqk_c = np.concatenate([bq_c, bk_c])[None, :].astype(bf16)
        bias_means = np.array(
            [bq_c[h * HD:(h + 1) * HD].mean() for h in range(HC)]
            + [bk_c[h * HD:(h + 1) * HD].mean() for h in range(HC)],
            np.float32)
        bvm_c = np.concatenate([bv_c, bias_means])[None, :].astype(bf16)
        in_maps.append({
            "xT": xTb[bt],
            "wqkT": np.ascontiguousarray(wqkT),
            "bqk": np.ascontiguousarray(bqk_c),
            "wvmT": np.ascontiguousarray(wvmT),
            "bvm": np.ascontiguousarray(bvm_c),
            "wpT": np.ascontiguousarray(Wp[sl].T).astype(bf16),
            "bp": np.ascontiguousarray(bpv[sl].reshape(2 * P, 1)),
            "qg2": np.tile(qg, 2).reshape(P, 1).astype(np.float32),
            "qb2": np.tile(qb, 2).reshape(P, 1).astype(np.float32),
            "kg2": np.tile(kg, 2).reshape(P, 1).astype(np.float32),
            "kb2": np.tile(kb, 2).reshape(P, 1).astype(np.float32),
            "ident": identity,
        })
    return in_maps, trivial_gb


def assemble_output(results):
    out = np.empty((B, NSEQ, D), np.float32)
    for c in range(N_CORES):
        bt, l = divmod(c, L)
        o = np.asarray(results[c]["outT"], np.float32)   # [256, 2048]
        out[bt, :, l * 2 * P:(l + 1) * 2 * P] = o.T
    return out


_CACHE = {}


def kernel(**inputs):
    from concourse.bass_utils import run_bass_kernel_spmd

    in_maps, trivial = prep_inputs(inputs)
    key = ("nc", trivial)
    if key not in _CACHE:
        _CACHE[key] = build(trivial_gb=trivial)
    nc = _CACHE[key]
    res = run_bass_kernel_spmd(nc, in_maps, core_ids=list(range(N_CORES)))
    return assemble_output(res.results)
